# revision 45
# baseline (speedup 1.0000x reference)
"""Single-head causal attention (B=4, T=4096, E=1024, H=128) on 8 trn2 cores.

Sharding: core c -> (batch b = c//2, piece p = c%2). Within a batch the 32
query blocks of 128 rows are split even/odd between the two pieces so the
causal workload balances. SPMD: all per-core differences live in input data.

Device algorithm (per core, "transposed" layouts, weights pre-scaled by 4):
  All inputs are host-prearranged into partition-major layouts and loaded
  with a handful of large contiguous DMAs at program start.
  Projections: QT/KT/VT = W @ x^T.  Token tile 0 and query tile 0 run in
  f16 (protects early causal rows whose outputs don't average quantization
  noise); the rest are fp8e4 DoubleRow matmuls (256-deep contraction per
  pass, 2x PE throughput).  V is transposed to [tok, h] blocks on the PE
  and mirrored to fp8 via a gpsimd casting DMA.
  Attention per q-tile (512 queries) walks KEY-BLOCK PAIRS (2x128 keys):
    ST pair [128k, 2, 512q] = two f16 matmuls into one 2-bank PSUM tile
    PT = exp(scale*ST + ln(1/4))  (one ACT instruction per pair)
    below-diagonal pairs: PT in fp8 -> PV and l row-sum as DoubleRow matmuls
    diagonal-strip pairs: PT in f16 -> 2 f16 PV matmuls; PT accumulated into
      PTS (DVE) and reduced by one f16 matmul per tile
  Output: OT [h, 512] f32 and l per tile, normalized + transposed on the
  host (out = (OT/l).T / 4: the 4 from the weight scaling of V).
The exp prescale 1/4 keeps fp8 PT under the e4m3 max of 240; it cancels in
O/l.  The dual-fp8 LDWEIGHTS path needs the two stationary k-tiles >=16B
apart, hence the ones8 [P,2,16] padding (l lands on PSUM rows 0-15).
"""

import numpy as np
import ml_dtypes

B, T, E, H = 4, 4096, 1024, 128
P = 128
NB_E = E // P           # 8 contraction chunks of 128
TQ = T // 2             # 2048 gathered queries per core
N_QT = TQ // 512        # 4 q-tiles per core
WSC = 4.0               # weight pre-scale (host); scores scale by WSC^2
SCALE_ACT = float(H) ** -0.5 / (WSC * WSC)
LN_QUARTER = float(np.log(0.25))
NEG = -30000.0
N_CORES = 8
F32 = np.float32
F8NP = ml_dtypes.float8_e4m3
W8 = T - 512            # fp8 token columns
WQ8 = TQ - 512          # fp8 gathered-query columns


def _query_rows(p: int) -> np.ndarray:
    """Absolute row indices of the gathered queries for piece p (in order)."""
    blocks = [np.arange(256 * g + 128 * p, 256 * g + 128 * p + 128) for g in range(16)]
    return np.concatenate(blocks)


def _mask_compact(p: int) -> np.ndarray:
    """Compact causal mask [128, 8, 128] f16 (partition-major): plane j holds
    the additive mask for in-strip key block j at query columns
    [c0_j, c0_j+128), c0_j = 128*(j//2)."""
    out = np.empty((128, 8, 128), dtype=np.float16)
    for j in range(8):
        kk = np.arange(128)[:, None] + 128 * j
        q = np.arange(128)[None, :] + 128 * (j // 2)
        i, r = q // 128, q % 128
        visible = kk <= 256 * i + 128 * p + r
        out[:, j, :] = np.where(visible, 0.0, NEG)
    return out


def _emit(tc, aps):
    import concourse.bass as bass
    from concourse import mybir
    from concourse.masks import make_identity

    nc = tc.nc
    f32 = mybir.dt.float32
    f16 = mybir.dt.float16
    f8 = mybir.dt.float8e4
    EXP = mybir.ActivationFunctionType.Exp
    DR = mybir.MatmulPerfMode.DoubleRow

    (x16p, xq16p, x8p, xq8p, w16p, w8p, maskp, oT, lsum) = aps

    from contextlib import ExitStack

    ctx = ExitStack()
    with ctx:
        # ---- pools ----
        consts = ctx.enter_context(tc.tile_pool(name="consts", bufs=1))
        vt_pool = ctx.enter_context(tc.tile_pool(name="vt", bufs=2))
        pt8_pool = ctx.enter_context(tc.tile_pool(name="pt8", bufs=3))
        pt16_pool = ctx.enter_context(tc.tile_pool(name="pt16", bufs=3))
        pts_pool = ctx.enter_context(tc.tile_pool(name="pts", bufs=2))
        ptt_pool = ctx.enter_context(tc.tile_pool(name="ptt", bufs=2))
        osb_pool = ctx.enter_context(tc.tile_pool(name="osb", bufs=2))
        s_ps = ctx.enter_context(tc.tile_pool(name="sps", bufs=2, space="PSUM"))
        o_ps = ctx.enter_context(tc.tile_pool(name="ops", bufs=1, space="PSUM"))
        l_ps = ctx.enter_context(tc.tile_pool(name="lps", bufs=1, space="PSUM"))
        t_ps = ctx.enter_context(tc.tile_pool(name="tps", bufs=2, space="PSUM"))

        # ---- persistent SBUF tensors ----
        identity = consts.tile([P, P], f16)
        ones16 = consts.tile([P, 16], f16)
        ones8 = consts.tile([P, 2, 16], f8)
        bias_sb = consts.tile([P, 1], f32)
        x16_sb = [consts.tile([P, 2, 512], f16, name=f"x16_{i}")
                  for i in range(4)]
        xq16_sb = [consts.tile([P, 2, 512], f16, name=f"xq16_{i}")
                   for i in range(4)]
        # fp8 x tiles are split per round-range so each projection's
        # dependency covers exactly one DMA (deps are tile-granular)
        x8a_sb = [consts.tile([P, 2, 512], f8, name=f"x8a_{i}")
                  for i in range(4)]
        x8b_sb = [consts.tile([P, 2, 1024], f8, name=f"x8b_{i}")
                  for i in range(4)]
        x8c_sb = [consts.tile([P, 2, 2048], f8, name=f"x8c_{i}")
                  for i in range(4)]
        xq8a_sb = [consts.tile([P, 2, 512], f8, name=f"xq8a_{i}")
                   for i in range(4)]
        xq8b_sb = [consts.tile([P, 2, 1024], f8, name=f"xq8b_{i}")
                   for i in range(4)]
        w16q_sb = consts.tile([P, 1, NB_E, P], f16)
        w16kv_sb = consts.tile([P, 2, NB_E, P], f16)
        w8_sb = consts.tile([P, 3, NB_E, P], f8)
        mask_sb = consts.tile([P, 8, P], f16)
        kt_all = consts.tile([P, T], f16)
        l_all = consts.tile([1, N_QT, 512], f32)
        qt_all = consts.tile([P, TQ], f16)
        v16 = consts.tile([P, T // P, P], f16)
        v8 = consts.tile([P, T // P, P], f8)

        make_identity(nc, identity[:])
        nc.gpsimd.memset(ones16[:], 1.0)
        nc.gpsimd.memset(ones8[:], 1.0)
        nc.gpsimd.memset(bias_sb[:], LN_QUARTER)
        # DMA order tracks first use: Wq + xq16 unblock the very first
        # matmul; each round's fp8 ranges stream in just ahead of use.
        # Issuance is split across the two HWDGE rings: sync carries the
        # critical-path loads; the ACT sequencer (idle until the first exp)
        # issues the f16 K/V inputs and late fp8 bulk in parallel.
        nc.sync.dma_start(w16q_sb[:], w16p[:, 0:1, :, :])
        nc.sync.dma_start(xq16_sb[0][:], xq16p[:, 0:2, :])
        for h in range(1, 4):
            nc.scalar.dma_start(xq16_sb[h][:], xq16p[:, 2 * h:2 * h + 2, :])
        nc.scalar.dma_start(w16kv_sb[:], w16p[:, 1:3, :, :])
        for h in range(4):
            nc.scalar.dma_start(x16_sb[h][:], x16p[:, 2 * h:2 * h + 2, :])
        nc.scalar.dma_start(w8_sb[:], w8p[:, :, :, :])
        nc.sync.dma_start(mask_sb[:], maskp[:, :, :])
        for c2 in range(4):     # tok tile 1 (projected in round 0)
            nc.sync.dma_start(x8a_sb[c2][:], x8p[c2, :, :, 0:512])
        for c2 in range(4):     # q-tile 1 (projected during attention 0)
            nc.sync.dma_start(xq8a_sb[c2][:], xq8p[c2, :, :, 0:512])
        for c2 in range(4):     # tok tiles 2,3
            nc.sync.dma_start(x8b_sb[c2][:], x8p[c2, :, :, 512:1536])
        for c2 in range(4):     # q-tiles 2,3
            nc.sync.dma_start(xq8b_sb[c2][:], xq8p[c2, :, :, 512:1536])
        for c2 in range(4):     # tok tiles 4..7
            nc.sync.dma_start(x8c_sb[c2][:], x8p[c2, :, :, 1536:3584])

        def proj16(iw, x_sb, ps):
            w_sb = w16q_sb if iw == 0 else w16kv_sb
            jw = 0 if iw == 0 else iw - 1
            for c in range(NB_E):
                nc.tensor.matmul(
                    ps[:], lhsT=w_sb[:, jw, c, :],
                    rhs=x_sb[c // 2][:, c % 2, :],
                    start=(c == 0), stop=(c == NB_E - 1))

        def x8_range(tok):
            # (tile list, local column offset) for fp8 token tile tok
            if tok == 1:
                return x8a_sb, 0
            if tok < 4:
                return x8b_sb, (tok - 2) * 512
            return x8c_sb, (tok - 4) * 512

        def xq8_range(tt):
            if tt == 1:
                return xq8a_sb, 0
            return xq8b_sb, (tt - 2) * 512

        def proj8(iw, src_sb, off, ps):
            for c2 in range(4):
                nc.tensor.matmul(
                    ps[:], lhsT=w8_sb[:, iw, 2 * c2:2 * c2 + 2, :],
                    rhs=src_sb[c2][:, :, off:off + 512],
                    start=(c2 == 0), stop=(c2 == 3), perf_mode=DR)

        def v_chain(ps, tok):
            """PSUM VT [128h, 512tok] -> v16/v8 [tok, h] blocks."""
            vt = vt_pool.tile([P, 512], f16, tag="vt")
            nc.vector.tensor_copy(vt[:], ps[:])
            for u in range(4):
                kb = tok * 4 + u
                tp = t_ps.tile([P, P], f16, tag="tps", name=f"tp_{kb}")
                nc.tensor.transpose(tp[:], vt[:, u * P:(u + 1) * P], identity[:])
                nc.vector.tensor_copy(v16[:, kb, :], tp[:])
            nc.gpsimd.dma_start(v8[:, tok * 4:tok * 4 + 4, :],
                                v16[:, tok * 4:tok * 4 + 4, :])

        def q_unit(tt):
            tiles, off = xq8_range(tt)
            qps = s_ps.tile([P, 512], f32, tag="sps", name=f"qp_{tt}")
            proj8(0, tiles, off, qps)
            nc.vector.tensor_copy(qt_all[:, tt * 512:(tt + 1) * 512], qps[:])

        def k_unit(tok):
            tiles, off = x8_range(tok)
            kps = s_ps.tile([P, 512], f32, tag="sps", name=f"kp_{tok}")
            proj8(1, tiles, off, kps)
            nc.vector.tensor_copy(kt_all[:, tok * 512:(tok + 1) * 512], kps[:])

        def v_unit(tok):
            tiles, off = x8_range(tok)
            vps = s_ps.tile([P, 512], f32, tag="sps", name=f"vp_{tok}")
            proj8(2, tiles, off, vps)
            v_chain(vps, tok)

        def proj_units(tt):
            """Projection work for round tt (interleaved into attention
            of round tt-1): q-tile tt, token tiles 2tt, 2tt+1."""
            if tt >= N_QT:
                return []
            return [lambda: q_unit(tt),
                    lambda: k_unit(2 * tt), lambda: v_unit(2 * tt),
                    lambda: k_unit(2 * tt + 1), lambda: v_unit(2 * tt + 1)]

        # ---- round 0 projections (f16 fixup path + fp8 token tile 1) ----
        qps = s_ps.tile([P, 512], f32, tag="sps", name="qp_0")
        proj16(0, xq16_sb, qps)
        nc.vector.tensor_copy(qt_all[:, 0:512], qps[:])
        kps = s_ps.tile([P, 512], f32, tag="sps", name="kp_0")
        proj16(1, x16_sb, kps)
        nc.vector.tensor_copy(kt_all[:, 0:512], kps[:])
        vps = s_ps.tile([P, 512], f32, tag="sps", name="vp_0")
        proj16(2, x16_sb, vps)
        v_chain(vps, 0)

        # ---- rounds: attention tt with round tt+1's projections woven in
        # (token tile 1 rides in attention 0's queue: its kt/v blocks are
        # first read by attention 0's pairs 2-3, after the unit completes)
        for tt in range(N_QT):
            units = proj_units(tt + 1)
            if tt == 0:
                units = [lambda: k_unit(1), lambda: v_unit(1)] + units

            # ---- attention for q-tile tt ----
            qs = qt_all[:, tt * 512:(tt + 1) * 512]
            ot = o_ps.tile([P, 512], f32, tag="ops", name=f"ot_{tt}")
            lt = l_ps.tile([16, 512], f32, tag="lps", name=f"lt_{tt}")
            pts = pts_pool.tile([P, 512], f16, tag="pts", name=f"pts_{tt}")
            npair = 4 * tt + 4
            nbelow = 4 * tt
            s_tiles = [None] * npair

            def c0_of(u):
                return 0 if u < nbelow else 128 * (u - nbelow)

            def emit_scores(u):
                c0 = c0_of(u)
                s = s_ps.tile([P, 2, 512], f32, tag="sps", name=f"s_{tt}_{u}")
                for m in (0, 1):
                    kb = 2 * u + m
                    nc.tensor.matmul(
                        s[:, m, c0:512],
                        lhsT=kt_all[:, kb * P:(kb + 1) * P],
                        rhs=qs[:, c0:512], start=True, stop=True)
                if u >= nbelow:
                    d = u - nbelow
                    nc.vector.tensor_add(
                        s[:, :, c0:c0 + P], s[:, :, c0:c0 + P],
                        mask_sb[:, 2 * d:2 * d + 2, :])
                s_tiles[u] = s

            emit_scores(0)
            for u in range(npair):
                if u + 1 < npair:
                    emit_scores(u + 1)
                s = s_tiles[u]
                c0 = c0_of(u)
                if u < nbelow or tt == N_QT - 1:
                    # fp8 path; the last tile's diagonal also runs fp8
                    # (its rows are global >=3072 so quantization washes
                    # out) which drops the PTS chain from the kernel tail
                    d = u - nbelow
                    last = (tt == N_QT - 1 and u == npair - 1)
                    pt = pt8_pool.tile([P, 2, 512], f8, tag="pt8")
                    nc.scalar.activation(pt[:, :, c0:512], s[:, :, c0:512], EXP,
                                         bias=bias_sb[:], scale=SCALE_ACT)
                    kb2 = 2 * u if u < nbelow else 8 * tt + 2 * d
                    nc.tensor.matmul(
                        ot[:, c0:512], lhsT=v8[:, kb2:kb2 + 2, :],
                        rhs=pt[:, :, c0:512],
                        start=(u == 0), stop=last, perf_mode=DR)
                    nc.tensor.matmul(
                        lt[:, c0:512], lhsT=ones8[:], rhs=pt[:, :, c0:512],
                        start=(u == 0), stop=last, perf_mode=DR)
                else:
                    d = u - nbelow
                    pt = pt16_pool.tile([P, 2, 512], f16, tag="pt16")
                    nc.scalar.activation(pt[:, :, c0:512], s[:, :, c0:512], EXP,
                                         bias=bias_sb[:], scale=SCALE_ACT)
                    for m in (0, 1):
                        nc.tensor.matmul(
                            ot[:, c0:512],
                            lhsT=v16[:, 8 * tt + 2 * d + m, :],
                            rhs=pt[:, m, c0:512],
                            start=(u == 0 and m == 0),
                            stop=(u == npair - 1 and m == 1))
                    if d == 0:
                        nc.vector.tensor_add(pts[:], pt[:, 0, :], pt[:, 1, :])
                    else:
                        tmp = ptt_pool.tile([P, 512], f16, tag="ptt")
                        nc.vector.tensor_add(tmp[:, c0:512], pt[:, 0, c0:512],
                                             pt[:, 1, c0:512])
                        nc.vector.tensor_add(pts[:, c0:512], pts[:, c0:512],
                                             tmp[:, c0:512])
                if units:
                    units.pop(0)()
            while units:
                units.pop(0)()
            if tt < N_QT - 1:
                nc.tensor.matmul(lt[:], lhsT=ones16[:], rhs=pts[:],
                                 start=(tt == 0), stop=True)

            # epilogue: PSUM -> SBUF -> HBM (normalize + transpose on host)
            o_sb = osb_pool.tile([P, 512], f32, tag="osb")
            nc.vector.tensor_copy(o_sb[:], ot[:])
            nc.vector.tensor_copy(l_all[0:1, tt, :], lt[0:1, :])
            nc.sync.dma_start(oT[tt, :, :], o_sb[:])
        nc.sync.dma_start(lsum[:, :], l_all[0:1, :, :])


def build_program():
    import concourse.tile as tile
    from concourse import bacc, mybir

    f32 = mybir.dt.float32
    f16 = mybir.dt.float16
    f8 = mybir.dt.float8e4
    nc = bacc.Bacc("TRN2", target_bir_lowering=False, debug=False,
                   num_devices=N_CORES)
    x16p = nc.dram_tensor("x16p", [P, NB_E, 512], f16, kind="ExternalInput").ap()
    xq16p = nc.dram_tensor("xq16p", [P, NB_E, 512], f16, kind="ExternalInput").ap()
    x8p = nc.dram_tensor("x8p", [4, P, 2, W8], f8, kind="ExternalInput").ap()
    xq8p = nc.dram_tensor("xq8p", [4, P, 2, WQ8], f8, kind="ExternalInput").ap()
    w16p = nc.dram_tensor("w16p", [P, 3, NB_E, P], f16, kind="ExternalInput").ap()
    w8p = nc.dram_tensor("w8p", [P, 3, NB_E, P], f8, kind="ExternalInput").ap()
    maskp = nc.dram_tensor("maskp", [P, 8, P], f16, kind="ExternalInput").ap()
    oT = nc.dram_tensor("oT", [N_QT, P, 512], f32, kind="ExternalOutput").ap()
    lsum = nc.dram_tensor("lsum", [N_QT, 512], f32, kind="ExternalOutput").ap()

    with tile.TileContext(nc) as tc:
        _emit(tc, (x16p, xq16p, x8p, xq8p, w16p, w8p, maskp, oT, lsum))
    nc.compile()
    return nc


def make_in_maps(x, Wq, Wk, Wv):
    """Per-core input maps. x: [B,T,E] f32; W*: [H,E] f32."""
    x = np.asarray(x, dtype=F32)
    # weights: [E, H] scaled, partition-major [P, 3, NB_E, P]
    w16p = np.empty((P, 3, NB_E, P), dtype=np.float16)
    w8p = np.empty((P, 3, NB_E, P), dtype=F8NP)
    for iw, W in enumerate((Wq, Wk, Wv)):
        wt = (np.asarray(W, dtype=F32).T * WSC)          # [E, H]
        wt = wt.reshape(NB_E, P, H).transpose(1, 0, 2)   # [P, NB_E, H]
        w16p[:, iw] = wt.astype(np.float16)
        w8p[:, iw] = wt.astype(F8NP)

    def pair_pack(arr, width):
        """[E, width] -> [4, P, 2, width]: chunk-pair partition-major."""
        return np.ascontiguousarray(
            arr.reshape(4, 2, P, width).transpose(0, 2, 1, 3))

    in_maps = []
    for c in range(N_CORES):
        b, p = c // 2, c % 2
        xb = x[b]                                       # [T, E]
        xT = np.ascontiguousarray(xb.T)                 # [E, T]
        xq = np.ascontiguousarray(xb[_query_rows(p)].T)  # [E, TQ]
        x16 = xT[:, :512].reshape(NB_E, P, 512)
        xq16 = xq[:, :512].reshape(NB_E, P, 512)
        in_maps.append({
            "x16p": np.ascontiguousarray(x16.transpose(1, 0, 2).astype(np.float16)),
            "xq16p": np.ascontiguousarray(xq16.transpose(1, 0, 2).astype(np.float16)),
            "x8p": pair_pack(np.ascontiguousarray(xT[:, 512:]).astype(F8NP), W8),
            "xq8p": pair_pack(np.ascontiguousarray(xq[:, 512:]).astype(F8NP), WQ8),
            "w16p": w16p, "w8p": w8p,
            "maskp": _mask_compact(p),
        })
    return in_maps


def postprocess(core_out):
    """Device outputs -> [TQ, H] f32 in gathered-row order."""
    oT = np.asarray(core_out["oT"], dtype=F32)      # [4, 128, 512]
    l = np.asarray(core_out["lsum"], dtype=F32)     # [4, 512]
    out = np.empty((TQ, H), dtype=F32)
    for t in range(N_QT):
        out[t * 512:(t + 1) * 512] = (oT[t] / l[t][None, :]).T / WSC
    return out


def _enable_ldw_opt():
    """Walrus ships with --enable-ldw-opt=false; the optimization overlaps
    LDWEIGHTS with the preceding matmul stream (weight double-buffering),
    which otherwise serializes ~100ns per matmul on the PE."""
    import concourse.bass_utils as bu
    if getattr(bu, "_ldw_patched", False):
        return
    bu._ldw_patched = True  # ldw-opt is incompatible with explicit
    # Ldweights codegen in this toolchain (walrus rejects it); LDWEIGHTS
    # already overlaps matmuls via the weight double-buffer.


def run(x, Wq, Wk, Wv, trace=False, trace_cores=None):
    """Returns (full_output [B,T,H] f32, BassKernelResults)."""
    from concourse.bass_utils import run_bass_kernel_spmd

    _enable_ldw_opt()
    nc = build_program()
    in_maps = make_in_maps(x, Wq, Wk, Wv)
    res = run_bass_kernel_spmd(
        nc, in_maps, list(range(N_CORES)), trace=trace,
        trace_cores=trace_cores,
    )
    full = np.empty((B, T, H), dtype=F32)
    for c in range(N_CORES):
        b, p = c // 2, c % 2
        full[b, _query_rows(p), :] = postprocess(res.results[c])
    return full, res


def kernel(x, Wq, Wk, Wv):
    full, _ = run(x, Wq, Wk, Wv, trace=False)
    return full


if __name__ == "__main__":
    nc = build_program()
    print("program built ok")


# revision 46
# speedup vs baseline: 1.0731x; 1.0731x over previous
"""Single-head causal attention (B=4, T=4096, E=1024, H=128) on 8 trn2 cores.

Sharding: core c -> (batch b = c//2, piece p = c%2). Within a batch the 32
query blocks of 128 rows are split even/odd between the two pieces so the
causal workload balances. SPMD: all per-core differences live in input data.

Device algorithm (per core, "transposed" layouts, weights pre-scaled by 4):
  All inputs are host-prearranged into partition-major layouts and loaded
  with a handful of large contiguous DMAs at program start.
  Projections: QT/KT/VT = W @ x^T.  Token tile 0 and query tile 0 run in
  f16 (protects early causal rows whose outputs don't average quantization
  noise); the rest are fp8e4 DoubleRow matmuls (256-deep contraction per
  pass, 2x PE throughput).  V is transposed to [tok, h] blocks on the PE
  and mirrored to fp8 via a gpsimd casting DMA.
  Attention per q-tile (512 queries) walks KEY-BLOCK PAIRS (2x128 keys):
    ST pair [128k, 2, 512q] = two f16 matmuls into one 2-bank PSUM tile
    PT = exp(scale*ST + ln(1/4))  (one ACT instruction per pair)
    below-diagonal pairs: PT in fp8 -> PV and l row-sum as DoubleRow matmuls
    diagonal-strip pairs: PT in f16 -> 2 f16 PV matmuls; PT accumulated into
      PTS (DVE) and reduced by one f16 matmul per tile
  Output: OT [h, 512] f32 and l per tile, normalized + transposed on the
  host (out = (OT/l).T / 4: the 4 from the weight scaling of V).
The exp prescale 1/4 keeps fp8 PT under the e4m3 max of 240; it cancels in
O/l.  The dual-fp8 LDWEIGHTS path needs the two stationary k-tiles >=16B
apart, hence the ones8 [P,2,16] padding (l lands on PSUM rows 0-15).
"""

import numpy as np
import ml_dtypes

B, T, E, H = 4, 4096, 1024, 128
P = 128
NB_E = E // P           # 8 contraction chunks of 128
TQ = T // 2             # 2048 gathered queries per core
N_QT = TQ // 512        # 4 q-tiles per core
WSC = 4.0               # weight pre-scale (host); scores scale by WSC^2
SCALE_ACT = float(H) ** -0.5 / (WSC * WSC)
LN_QUARTER = float(np.log(0.25))
NEG = -30000.0
N_CORES = 8
F32 = np.float32
F8NP = ml_dtypes.float8_e4m3
W8 = T - 512            # fp8 token columns
WQ8 = TQ - 512          # fp8 gathered-query columns


def _query_rows(p: int) -> np.ndarray:
    """Absolute row indices of the gathered queries for piece p (in order)."""
    blocks = [np.arange(256 * g + 128 * p, 256 * g + 128 * p + 128) for g in range(16)]
    return np.concatenate(blocks)


def _mask_compact(p: int) -> np.ndarray:
    """Compact causal mask [128, 8, 128] f16 (partition-major): plane j holds
    the additive mask for in-strip key block j at query columns
    [c0_j, c0_j+128), c0_j = 128*(j//2)."""
    out = np.empty((128, 8, 128), dtype=np.float16)
    for j in range(8):
        kk = np.arange(128)[:, None] + 128 * j
        q = np.arange(128)[None, :] + 128 * (j // 2)
        i, r = q // 128, q % 128
        visible = kk <= 256 * i + 128 * p + r
        out[:, j, :] = np.where(visible, 0.0, NEG)
    return out


def _emit(tc, aps):
    import concourse.bass as bass
    from concourse import mybir
    from concourse.masks import make_identity

    nc = tc.nc
    f32 = mybir.dt.float32
    f16 = mybir.dt.float16
    f8 = mybir.dt.float8e4
    EXP = mybir.ActivationFunctionType.Exp
    DR = mybir.MatmulPerfMode.DoubleRow

    (x16p, xq16p, x8p, xq8p, w16p, w8p, maskp, oT, lsum) = aps

    from contextlib import ExitStack

    ctx = ExitStack()
    with ctx:
        # ---- pools ----
        consts = ctx.enter_context(tc.tile_pool(name="consts", bufs=1))
        vt_pool = ctx.enter_context(tc.tile_pool(name="vt", bufs=2))
        pt8_pool = ctx.enter_context(tc.tile_pool(name="pt8", bufs=3))
        pt16_pool = ctx.enter_context(tc.tile_pool(name="pt16", bufs=3))
        pts_pool = ctx.enter_context(tc.tile_pool(name="pts", bufs=2))
        ptt_pool = ctx.enter_context(tc.tile_pool(name="ptt", bufs=2))
        osb_pool = ctx.enter_context(tc.tile_pool(name="osb", bufs=2))
        s_ps = ctx.enter_context(tc.tile_pool(name="sps", bufs=2, space="PSUM"))
        o_ps = ctx.enter_context(tc.tile_pool(name="ops", bufs=1, space="PSUM"))
        l_ps = ctx.enter_context(tc.tile_pool(name="lps", bufs=1, space="PSUM"))
        t_ps = ctx.enter_context(tc.tile_pool(name="tps", bufs=2, space="PSUM"))

        # ---- persistent SBUF tensors ----
        identity = consts.tile([P, P], f16)
        ones16 = consts.tile([P, 16], f16)
        ones8 = consts.tile([P, 2, 16], f8)
        bias_sb = consts.tile([P, 1], f32)
        x16_sb = [consts.tile([P, 2, 512], f16, name=f"x16_{i}")
                  for i in range(4)]
        xq16_sb = [consts.tile([P, 2, 512], f16, name=f"xq16_{i}")
                   for i in range(4)]
        # fp8 x tiles are split per round-range so each projection's
        # dependency covers exactly one DMA (deps are tile-granular)
        x8a_sb = [consts.tile([P, 2, 512], f8, name=f"x8a_{i}")
                  for i in range(4)]
        x8b_sb = [consts.tile([P, 2, 1024], f8, name=f"x8b_{i}")
                  for i in range(4)]
        x8c_sb = [consts.tile([P, 2, 2048], f8, name=f"x8c_{i}")
                  for i in range(4)]
        xq8a_sb = [consts.tile([P, 2, 512], f8, name=f"xq8a_{i}")
                   for i in range(4)]
        xq8b_sb = [consts.tile([P, 2, 1024], f8, name=f"xq8b_{i}")
                   for i in range(4)]
        w16q_sb = consts.tile([P, 1, NB_E, P], f16)
        w16kv_sb = consts.tile([P, 2, NB_E, P], f16)
        w8_sb = consts.tile([P, 3, NB_E, P], f8)
        mask_sb = consts.tile([P, 8, P], f16)
        kt_all = consts.tile([P, T], f16)
        l_all = consts.tile([1, N_QT, 512], f32)
        qt_all = consts.tile([P, TQ], f16)
        v16 = consts.tile([P, T // P, P], f16)
        v8 = consts.tile([P, T // P, P], f8)

        make_identity(nc, identity[:])
        nc.gpsimd.memset(ones16[:], 1.0)
        nc.gpsimd.memset(ones8[:], 1.0)
        nc.gpsimd.memset(bias_sb[:], LN_QUARTER)
        # DMA order tracks first use: Wq + xq16 unblock the very first
        # matmul; each round's fp8 ranges stream in just ahead of use.
        # Issuance is split across the two HWDGE rings: sync carries the
        # critical-path loads; the ACT sequencer (idle until the first exp)
        # issues the f16 K/V inputs and late fp8 bulk in parallel.
        nc.sync.dma_start(w16q_sb[:], w16p[:, 0:1, :, :])
        for h in range(2):
            nc.sync.dma_start(xq16_sb[h][:], xq16p[:, 2 * h:2 * h + 2, :])
        for h in range(2, 4):
            nc.scalar.dma_start(xq16_sb[h][:], xq16p[:, 2 * h:2 * h + 2, :])
        nc.scalar.dma_start(w16kv_sb[:], w16p[:, 1:3, :, :])
        for h in range(4):
            nc.scalar.dma_start(x16_sb[h][:], x16p[:, 2 * h:2 * h + 2, :])
        nc.scalar.dma_start(w8_sb[:], w8p[:, :, :, :])
        nc.sync.dma_start(mask_sb[:], maskp[:, :, :])
        for c2 in range(4):     # tok tile 1 (projected in round 0)
            nc.sync.dma_start(x8a_sb[c2][:], x8p[c2, :, :, 0:512])
        for c2 in range(4):     # q-tile 1 (projected during attention 0)
            nc.sync.dma_start(xq8a_sb[c2][:], xq8p[c2, :, :, 0:512])
        for c2 in range(4):     # tok tiles 2,3
            nc.sync.dma_start(x8b_sb[c2][:], x8p[c2, :, :, 512:1536])
        for c2 in range(4):     # q-tiles 2,3
            nc.sync.dma_start(xq8b_sb[c2][:], xq8p[c2, :, :, 512:1536])
        for c2 in range(4):     # tok tiles 4..7
            nc.sync.dma_start(x8c_sb[c2][:], x8p[c2, :, :, 1536:3584])

        def proj16(iw, x_sb, ps):
            w_sb = w16q_sb if iw == 0 else w16kv_sb
            jw = 0 if iw == 0 else iw - 1
            for c in range(NB_E):
                nc.tensor.matmul(
                    ps[:], lhsT=w_sb[:, jw, c, :],
                    rhs=x_sb[c // 2][:, c % 2, :],
                    start=(c == 0), stop=(c == NB_E - 1))

        def x8_range(tok):
            # (tile list, local column offset) for fp8 token tile tok
            if tok == 1:
                return x8a_sb, 0
            if tok < 4:
                return x8b_sb, (tok - 2) * 512
            return x8c_sb, (tok - 4) * 512

        def xq8_range(tt):
            if tt == 1:
                return xq8a_sb, 0
            return xq8b_sb, (tt - 2) * 512

        def proj8(iw, src_sb, off, ps):
            for c2 in range(4):
                nc.tensor.matmul(
                    ps[:], lhsT=w8_sb[:, iw, 2 * c2:2 * c2 + 2, :],
                    rhs=src_sb[c2][:, :, off:off + 512],
                    start=(c2 == 0), stop=(c2 == 3), perf_mode=DR)

        def v_chain(ps, tok):
            """PSUM VT [128h, 512tok] -> v16/v8 [tok, h] blocks."""
            vt = vt_pool.tile([P, 512], f16, tag="vt")
            nc.vector.tensor_copy(vt[:], ps[:])
            for u in range(4):
                kb = tok * 4 + u
                tp = t_ps.tile([P, P], f16, tag="tps", name=f"tp_{kb}")
                nc.tensor.transpose(tp[:], vt[:, u * P:(u + 1) * P], identity[:])
                nc.vector.tensor_copy(v16[:, kb, :], tp[:])
            nc.gpsimd.dma_start(v8[:, tok * 4:tok * 4 + 4, :],
                                v16[:, tok * 4:tok * 4 + 4, :])

        def q_unit(tt):
            tiles, off = xq8_range(tt)
            qps = s_ps.tile([P, 512], f32, tag="sps", name=f"qp_{tt}")
            proj8(0, tiles, off, qps)
            nc.vector.tensor_copy(qt_all[:, tt * 512:(tt + 1) * 512], qps[:])

        def k_unit(tok):
            tiles, off = x8_range(tok)
            kps = s_ps.tile([P, 512], f32, tag="sps", name=f"kp_{tok}")
            proj8(1, tiles, off, kps)
            nc.vector.tensor_copy(kt_all[:, tok * 512:(tok + 1) * 512], kps[:])

        def v_unit(tok):
            tiles, off = x8_range(tok)
            vps = s_ps.tile([P, 512], f32, tag="sps", name=f"vp_{tok}")
            proj8(2, tiles, off, vps)
            v_chain(vps, tok)

        def proj_units(tt):
            """Projection work for round tt (interleaved into attention
            of round tt-1): q-tile tt, token tiles 2tt, 2tt+1."""
            if tt >= N_QT:
                return []
            return [lambda: q_unit(tt),
                    lambda: k_unit(2 * tt), lambda: v_unit(2 * tt),
                    lambda: k_unit(2 * tt + 1), lambda: v_unit(2 * tt + 1)]

        # ---- round 0 projections (f16 fixup path + fp8 token tile 1) ----
        qps = s_ps.tile([P, 512], f32, tag="sps", name="qp_0")
        proj16(0, xq16_sb, qps)
        nc.vector.tensor_copy(qt_all[:, 0:512], qps[:])
        kps = s_ps.tile([P, 512], f32, tag="sps", name="kp_0")
        proj16(1, x16_sb, kps)
        nc.vector.tensor_copy(kt_all[:, 0:512], kps[:])
        vps = s_ps.tile([P, 512], f32, tag="sps", name="vp_0")
        proj16(2, x16_sb, vps)
        v_chain(vps, 0)

        # ---- rounds: attention tt with round tt+1's projections woven in
        # (token tile 1 rides in attention 0's queue: its kt/v blocks are
        # first read by attention 0's pairs 2-3, after the unit completes)
        for tt in range(N_QT):
            units = proj_units(tt + 1)
            if tt == 0:
                units = [lambda: k_unit(1), lambda: v_unit(1)] + units

            # ---- attention for q-tile tt ----
            qs = qt_all[:, tt * 512:(tt + 1) * 512]
            ot = o_ps.tile([P, 512], f32, tag="ops", name=f"ot_{tt}")
            lt = l_ps.tile([16, 512], f32, tag="lps", name=f"lt_{tt}")
            pts = pts_pool.tile([P, 512], f16, tag="pts", name=f"pts_{tt}")
            npair = 4 * tt + 4
            nbelow = 4 * tt
            s_tiles = [None] * npair

            def c0_of(u):
                return 0 if u < nbelow else 128 * (u - nbelow)

            def emit_scores(u):
                c0 = c0_of(u)
                s = s_ps.tile([P, 2, 512], f32, tag="sps", name=f"s_{tt}_{u}")
                for m in (0, 1):
                    kb = 2 * u + m
                    nc.tensor.matmul(
                        s[:, m, c0:512],
                        lhsT=kt_all[:, kb * P:(kb + 1) * P],
                        rhs=qs[:, c0:512], start=True, stop=True)
                if u >= nbelow:
                    d = u - nbelow
                    nc.vector.tensor_add(
                        s[:, :, c0:c0 + P], s[:, :, c0:c0 + P],
                        mask_sb[:, 2 * d:2 * d + 2, :])
                s_tiles[u] = s

            emit_scores(0)
            for u in range(npair):
                if u + 1 < npair:
                    emit_scores(u + 1)
                s = s_tiles[u]
                c0 = c0_of(u)
                if u < nbelow or tt == N_QT - 1:
                    # fp8 path; the last tile's diagonal also runs fp8
                    # (its rows are global >=3072 so quantization washes
                    # out) which drops the PTS chain from the kernel tail
                    d = u - nbelow
                    last = (tt == N_QT - 1 and u == npair - 1)
                    pt = pt8_pool.tile([P, 2, 512], f8, tag="pt8")
                    nc.scalar.activation(pt[:, :, c0:512], s[:, :, c0:512], EXP,
                                         bias=bias_sb[:], scale=SCALE_ACT)
                    kb2 = 2 * u if u < nbelow else 8 * tt + 2 * d
                    nc.tensor.matmul(
                        ot[:, c0:512], lhsT=v8[:, kb2:kb2 + 2, :],
                        rhs=pt[:, :, c0:512],
                        start=(u == 0), stop=last, perf_mode=DR)
                    nc.tensor.matmul(
                        lt[:, c0:512], lhsT=ones8[:], rhs=pt[:, :, c0:512],
                        start=(u == 0), stop=last, perf_mode=DR)
                else:
                    d = u - nbelow
                    pt = pt16_pool.tile([P, 2, 512], f16, tag="pt16")
                    nc.scalar.activation(pt[:, :, c0:512], s[:, :, c0:512], EXP,
                                         bias=bias_sb[:], scale=SCALE_ACT)
                    for m in (0, 1):
                        nc.tensor.matmul(
                            ot[:, c0:512],
                            lhsT=v16[:, 8 * tt + 2 * d + m, :],
                            rhs=pt[:, m, c0:512],
                            start=(u == 0 and m == 0),
                            stop=(u == npair - 1 and m == 1))
                    if d == 0:
                        nc.vector.tensor_add(pts[:], pt[:, 0, :], pt[:, 1, :])
                    else:
                        tmp = ptt_pool.tile([P, 512], f16, tag="ptt")
                        nc.vector.tensor_add(tmp[:, c0:512], pt[:, 0, c0:512],
                                             pt[:, 1, c0:512])
                        nc.vector.tensor_add(pts[:, c0:512], pts[:, c0:512],
                                             tmp[:, c0:512])
                if units:
                    units.pop(0)()
            while units:
                units.pop(0)()
            if tt < N_QT - 1:
                nc.tensor.matmul(lt[:], lhsT=ones16[:], rhs=pts[:],
                                 start=(tt == 0), stop=True)

            # epilogue: PSUM -> SBUF -> HBM (normalize + transpose on host)
            o_sb = osb_pool.tile([P, 512], f32, tag="osb")
            nc.vector.tensor_copy(o_sb[:], ot[:])
            nc.vector.tensor_copy(l_all[0:1, tt, :], lt[0:1, :])
            nc.sync.dma_start(oT[tt, :, :], o_sb[:])
        nc.sync.dma_start(lsum[:, :], l_all[0:1, :, :])


def build_program():
    import concourse.tile as tile
    from concourse import bacc, mybir

    f32 = mybir.dt.float32
    f16 = mybir.dt.float16
    f8 = mybir.dt.float8e4
    nc = bacc.Bacc("TRN2", target_bir_lowering=False, debug=False,
                   num_devices=N_CORES)
    x16p = nc.dram_tensor("x16p", [P, NB_E, 512], f16, kind="ExternalInput").ap()
    xq16p = nc.dram_tensor("xq16p", [P, NB_E, 512], f16, kind="ExternalInput").ap()
    x8p = nc.dram_tensor("x8p", [4, P, 2, W8], f8, kind="ExternalInput").ap()
    xq8p = nc.dram_tensor("xq8p", [4, P, 2, WQ8], f8, kind="ExternalInput").ap()
    w16p = nc.dram_tensor("w16p", [P, 3, NB_E, P], f16, kind="ExternalInput").ap()
    w8p = nc.dram_tensor("w8p", [P, 3, NB_E, P], f8, kind="ExternalInput").ap()
    maskp = nc.dram_tensor("maskp", [P, 8, P], f16, kind="ExternalInput").ap()
    oT = nc.dram_tensor("oT", [N_QT, P, 512], f32, kind="ExternalOutput").ap()
    lsum = nc.dram_tensor("lsum", [N_QT, 512], f32, kind="ExternalOutput").ap()

    with tile.TileContext(nc) as tc:
        _emit(tc, (x16p, xq16p, x8p, xq8p, w16p, w8p, maskp, oT, lsum))
    nc.compile()
    return nc


def make_in_maps(x, Wq, Wk, Wv):
    """Per-core input maps. x: [B,T,E] f32; W*: [H,E] f32."""
    x = np.asarray(x, dtype=F32)
    # weights: [E, H] scaled, partition-major [P, 3, NB_E, P]
    w16p = np.empty((P, 3, NB_E, P), dtype=np.float16)
    w8p = np.empty((P, 3, NB_E, P), dtype=F8NP)
    for iw, W in enumerate((Wq, Wk, Wv)):
        wt = (np.asarray(W, dtype=F32).T * WSC)          # [E, H]
        wt = wt.reshape(NB_E, P, H).transpose(1, 0, 2)   # [P, NB_E, H]
        w16p[:, iw] = wt.astype(np.float16)
        w8p[:, iw] = wt.astype(F8NP)

    def pair_pack(arr, width):
        """[E, width] -> [4, P, 2, width]: chunk-pair partition-major."""
        return np.ascontiguousarray(
            arr.reshape(4, 2, P, width).transpose(0, 2, 1, 3))

    in_maps = []
    for c in range(N_CORES):
        b, p = c // 2, c % 2
        xb = x[b]                                       # [T, E]
        xT = np.ascontiguousarray(xb.T)                 # [E, T]
        xq = np.ascontiguousarray(xb[_query_rows(p)].T)  # [E, TQ]
        x16 = xT[:, :512].reshape(NB_E, P, 512)
        xq16 = xq[:, :512].reshape(NB_E, P, 512)
        in_maps.append({
            "x16p": np.ascontiguousarray(x16.transpose(1, 0, 2).astype(np.float16)),
            "xq16p": np.ascontiguousarray(xq16.transpose(1, 0, 2).astype(np.float16)),
            "x8p": pair_pack(np.ascontiguousarray(xT[:, 512:]).astype(F8NP), W8),
            "xq8p": pair_pack(np.ascontiguousarray(xq[:, 512:]).astype(F8NP), WQ8),
            "w16p": w16p, "w8p": w8p,
            "maskp": _mask_compact(p),
        })
    return in_maps


def postprocess(core_out):
    """Device outputs -> [TQ, H] f32 in gathered-row order."""
    oT = np.asarray(core_out["oT"], dtype=F32)      # [4, 128, 512]
    l = np.asarray(core_out["lsum"], dtype=F32)     # [4, 512]
    out = np.empty((TQ, H), dtype=F32)
    for t in range(N_QT):
        out[t * 512:(t + 1) * 512] = (oT[t] / l[t][None, :]).T / WSC
    return out


def _enable_ldw_opt():
    """Walrus ships with --enable-ldw-opt=false; the optimization overlaps
    LDWEIGHTS with the preceding matmul stream (weight double-buffering),
    which otherwise serializes ~100ns per matmul on the PE."""
    import concourse.bass_utils as bu
    if getattr(bu, "_ldw_patched", False):
        return
    bu._ldw_patched = True  # ldw-opt is incompatible with explicit
    # Ldweights codegen in this toolchain (walrus rejects it); LDWEIGHTS
    # already overlaps matmuls via the weight double-buffer.


def run(x, Wq, Wk, Wv, trace=False, trace_cores=None):
    """Returns (full_output [B,T,H] f32, BassKernelResults)."""
    from concourse.bass_utils import run_bass_kernel_spmd

    _enable_ldw_opt()
    nc = build_program()
    in_maps = make_in_maps(x, Wq, Wk, Wv)
    res = run_bass_kernel_spmd(
        nc, in_maps, list(range(N_CORES)), trace=trace,
        trace_cores=trace_cores,
    )
    full = np.empty((B, T, H), dtype=F32)
    for c in range(N_CORES):
        b, p = c // 2, c % 2
        full[b, _query_rows(p), :] = postprocess(res.results[c])
    return full, res


def kernel(x, Wq, Wk, Wv):
    full, _ = run(x, Wq, Wk, Wv, trace=False)
    return full


if __name__ == "__main__":
    nc = build_program()
    print("program built ok")


# revision 49
# speedup vs baseline: 1.0810x; 1.0074x over previous
"""Single-head causal attention (B=4, T=4096, E=1024, H=128) on 8 trn2 cores.

Sharding: core c -> (batch b = c//2, piece p = c%2). Within a batch the 32
query blocks of 128 rows are split even/odd between the two pieces so the
causal workload balances. SPMD: all per-core differences live in input data.

Device algorithm (per core, "transposed" layouts, weights pre-scaled by 4):
  All inputs are host-prearranged into partition-major layouts and loaded
  with a handful of large contiguous DMAs at program start.
  Projections: QT/KT/VT = W @ x^T.  Token tile 0 and query tile 0 run in
  f16 (protects early causal rows whose outputs don't average quantization
  noise); the rest are fp8e4 DoubleRow matmuls (256-deep contraction per
  pass, 2x PE throughput).  V is transposed to [tok, h] blocks on the PE
  and mirrored to fp8 via a gpsimd casting DMA.
  Attention per q-tile (512 queries) walks KEY-BLOCK PAIRS (2x128 keys):
    ST pair [128k, 2, 512q] = two f16 matmuls into one 2-bank PSUM tile
    PT = exp(scale*ST + ln(1/4))  (one ACT instruction per pair)
    below-diagonal pairs: PT in fp8 -> PV and l row-sum as DoubleRow matmuls
    diagonal-strip pairs: PT in f16 -> 2 f16 PV matmuls; PT accumulated into
      PTS (DVE) and reduced by one f16 matmul per tile
  Output: OT [h, 512] f32 and l per tile, normalized + transposed on the
  host (out = (OT/l).T / 4: the 4 from the weight scaling of V).
The exp prescale 1/4 keeps fp8 PT under the e4m3 max of 240; it cancels in
O/l.  The dual-fp8 LDWEIGHTS path needs the two stationary k-tiles >=16B
apart, hence the ones8 [P,2,16] padding (l lands on PSUM rows 0-15).
"""

import numpy as np
import ml_dtypes

B, T, E, H = 4, 4096, 1024, 128
P = 128
NB_E = E // P           # 8 contraction chunks of 128
TQ = T // 2             # 2048 gathered queries per core
N_QT = TQ // 512        # 4 q-tiles per core
WSC = 4.0               # weight pre-scale (host); scores scale by WSC^2
SCALE_ACT = float(H) ** -0.5 / (WSC * WSC)
LN_QUARTER = float(np.log(0.25))
NEG = -30000.0
N_CORES = 8
F32 = np.float32
F8NP = ml_dtypes.float8_e4m3
W8 = T - 512            # fp8 token columns
WQ8 = TQ - 512          # fp8 gathered-query columns


def _query_rows(p: int) -> np.ndarray:
    """Absolute row indices of the gathered queries for piece p (in order)."""
    blocks = [np.arange(256 * g + 128 * p, 256 * g + 128 * p + 128) for g in range(16)]
    return np.concatenate(blocks)


def _mask_compact(p: int) -> np.ndarray:
    """Compact causal mask [128, 8, 128] f16 (partition-major): plane j holds
    the additive mask for in-strip key block j at query columns
    [c0_j, c0_j+128), c0_j = 128*(j//2)."""
    out = np.empty((128, 8, 128), dtype=np.float16)
    for j in range(8):
        kk = np.arange(128)[:, None] + 128 * j
        q = np.arange(128)[None, :] + 128 * (j // 2)
        i, r = q // 128, q % 128
        visible = kk <= 256 * i + 128 * p + r
        out[:, j, :] = np.where(visible, 0.0, NEG)
    return out


def _emit(tc, aps):
    import concourse.bass as bass
    from concourse import mybir
    from concourse.masks import make_identity

    nc = tc.nc
    f32 = mybir.dt.float32
    f16 = mybir.dt.float16
    f8 = mybir.dt.float8e4
    EXP = mybir.ActivationFunctionType.Exp
    DR = mybir.MatmulPerfMode.DoubleRow

    (x16p, xq16p, x8p, xq8p, w16p, w8p, maskp, oT, lsum) = aps

    from contextlib import ExitStack

    ctx = ExitStack()
    with ctx:
        # ---- pools ----
        consts = ctx.enter_context(tc.tile_pool(name="consts", bufs=1))
        vt_pool = ctx.enter_context(tc.tile_pool(name="vt", bufs=2))
        pt8_pool = ctx.enter_context(tc.tile_pool(name="pt8", bufs=3))
        pt16_pool = ctx.enter_context(tc.tile_pool(name="pt16", bufs=3))
        pts_pool = ctx.enter_context(tc.tile_pool(name="pts", bufs=2))
        ptt_pool = ctx.enter_context(tc.tile_pool(name="ptt", bufs=2))
        osb_pool = ctx.enter_context(tc.tile_pool(name="osb", bufs=2))
        s_ps = ctx.enter_context(tc.tile_pool(name="sps", bufs=2, space="PSUM"))
        o_ps = ctx.enter_context(tc.tile_pool(name="ops", bufs=1, space="PSUM"))
        l_ps = ctx.enter_context(tc.tile_pool(name="lps", bufs=1, space="PSUM"))
        t_ps = ctx.enter_context(tc.tile_pool(name="tps", bufs=2, space="PSUM"))

        # ---- persistent SBUF tensors ----
        identity = consts.tile([P, P], f16)
        ones16 = consts.tile([P, 16], f16)
        ones8 = consts.tile([P, 2, 16], f8)
        bias_sb = consts.tile([P, 1], f32)
        x16_sb = [consts.tile([P, 2, 512], f16, name=f"x16_{i}")
                  for i in range(4)]
        xq16_sb = [consts.tile([P, 2, 512], f16, name=f"xq16_{i}")
                   for i in range(4)]
        # fp8 x tiles are split per round-range so each projection's
        # dependency covers exactly one DMA (deps are tile-granular)
        x8a_sb = [consts.tile([P, 2, 512], f8, name=f"x8a_{i}")
                  for i in range(4)]
        x8b_sb = [consts.tile([P, 2, 1024], f8, name=f"x8b_{i}")
                  for i in range(4)]
        x8c_sb = [consts.tile([P, 2, 2048], f8, name=f"x8c_{i}")
                  for i in range(4)]
        xq8a_sb = [consts.tile([P, 2, 512], f8, name=f"xq8a_{i}")
                   for i in range(4)]
        xq8b_sb = [consts.tile([P, 2, 1024], f8, name=f"xq8b_{i}")
                   for i in range(4)]
        w16q_sb = [consts.tile([P, 1, 4, P], f16, name=f"w16q_{i}")
                   for i in range(2)]
        w16kv_sb = consts.tile([P, 2, NB_E, P], f16)
        w8_sb = consts.tile([P, 3, NB_E, P], f8)
        mask_sb = consts.tile([P, 8, P], f16)
        kt_all = consts.tile([P, T], f16)
        l_all = consts.tile([1, N_QT, 512], f32)
        qt_all = consts.tile([P, TQ], f16)
        v16 = consts.tile([P, T // P, P], f16)
        v8 = consts.tile([P, T // P, P], f8)

        make_identity(nc, identity[:])
        nc.gpsimd.memset(ones16[:], 1.0)
        nc.gpsimd.memset(ones8[:], 1.0)
        nc.gpsimd.memset(bias_sb[:], LN_QUARTER)
        # DMA order tracks first use: Wq + xq16 unblock the very first
        # matmul; each round's fp8 ranges stream in just ahead of use.
        # Issuance is split across the two HWDGE rings: sync carries the
        # critical-path loads; the ACT sequencer (idle until the first exp)
        # issues the f16 K/V inputs and late fp8 bulk in parallel.
        nc.sync.dma_start(w16q_sb[0][:], w16p[:, 0:1, 0:4, :])
        nc.sync.dma_start(xq16_sb[0][:], xq16p[:, 0:2, :])
        nc.sync.dma_start(w16q_sb[1][:], w16p[:, 0:1, 4:8, :])
        nc.sync.dma_start(xq16_sb[1][:], xq16p[:, 2:4, :])
        for h in range(2, 4):
            nc.scalar.dma_start(xq16_sb[h][:], xq16p[:, 2 * h:2 * h + 2, :])
        nc.scalar.dma_start(w16kv_sb[:], w16p[:, 1:3, :, :])
        for h in range(4):
            nc.scalar.dma_start(x16_sb[h][:], x16p[:, 2 * h:2 * h + 2, :])
        nc.scalar.dma_start(w8_sb[:], w8p[:, :, :, :])
        nc.sync.dma_start(mask_sb[:], maskp[:, :, :])
        for c2 in range(4):     # tok tile 1 (projected in round 0)
            nc.sync.dma_start(x8a_sb[c2][:], x8p[c2, :, :, 0:512])
        for c2 in range(4):     # q-tile 1 (projected during attention 0)
            nc.sync.dma_start(xq8a_sb[c2][:], xq8p[c2, :, :, 0:512])
        for c2 in range(4):     # tok tiles 2,3
            nc.sync.dma_start(x8b_sb[c2][:], x8p[c2, :, :, 512:1536])
        for c2 in range(4):     # q-tiles 2,3
            nc.sync.dma_start(xq8b_sb[c2][:], xq8p[c2, :, :, 512:1536])
        for c2 in range(4):     # tok tiles 4..7
            nc.sync.dma_start(x8c_sb[c2][:], x8p[c2, :, :, 1536:3584])

        def proj16(iw, x_sb, ps):
            for c in range(NB_E):
                if iw == 0:
                    w = w16q_sb[c // 4][:, 0, c % 4, :]
                else:
                    w = w16kv_sb[:, iw - 1, c, :]
                nc.tensor.matmul(
                    ps[:], lhsT=w, rhs=x_sb[c // 2][:, c % 2, :],
                    start=(c == 0), stop=(c == NB_E - 1))

        def x8_range(tok):
            # (tile list, local column offset) for fp8 token tile tok
            if tok == 1:
                return x8a_sb, 0
            if tok < 4:
                return x8b_sb, (tok - 2) * 512
            return x8c_sb, (tok - 4) * 512

        def xq8_range(tt):
            if tt == 1:
                return xq8a_sb, 0
            return xq8b_sb, (tt - 2) * 512

        def proj8(iw, src_sb, off, ps):
            for c2 in range(4):
                nc.tensor.matmul(
                    ps[:], lhsT=w8_sb[:, iw, 2 * c2:2 * c2 + 2, :],
                    rhs=src_sb[c2][:, :, off:off + 512],
                    start=(c2 == 0), stop=(c2 == 3), perf_mode=DR)

        def v_chain(ps, tok):
            """PSUM VT [128h, 512tok] -> v16/v8 [tok, h] blocks."""
            vt = vt_pool.tile([P, 512], f16, tag="vt")
            nc.vector.tensor_copy(vt[:], ps[:])
            for u in range(4):
                kb = tok * 4 + u
                tp = t_ps.tile([P, P], f16, tag="tps", name=f"tp_{kb}")
                nc.tensor.transpose(tp[:], vt[:, u * P:(u + 1) * P], identity[:])
                nc.vector.tensor_copy(v16[:, kb, :], tp[:])
            nc.gpsimd.dma_start(v8[:, tok * 4:tok * 4 + 4, :],
                                v16[:, tok * 4:tok * 4 + 4, :])

        def q_unit(tt):
            tiles, off = xq8_range(tt)
            qps = s_ps.tile([P, 512], f32, tag="sps", name=f"qp_{tt}")
            proj8(0, tiles, off, qps)
            nc.vector.tensor_copy(qt_all[:, tt * 512:(tt + 1) * 512], qps[:])

        def k_unit(tok):
            tiles, off = x8_range(tok)
            kps = s_ps.tile([P, 512], f32, tag="sps", name=f"kp_{tok}")
            proj8(1, tiles, off, kps)
            nc.vector.tensor_copy(kt_all[:, tok * 512:(tok + 1) * 512], kps[:])

        def v_unit(tok):
            tiles, off = x8_range(tok)
            vps = s_ps.tile([P, 512], f32, tag="sps", name=f"vp_{tok}")
            proj8(2, tiles, off, vps)
            v_chain(vps, tok)

        def proj_units(tt):
            """Projection work for round tt (interleaved into attention
            of round tt-1): q-tile tt, token tiles 2tt, 2tt+1."""
            if tt >= N_QT:
                return []
            return [lambda: q_unit(tt),
                    lambda: k_unit(2 * tt), lambda: v_unit(2 * tt),
                    lambda: k_unit(2 * tt + 1), lambda: v_unit(2 * tt + 1)]

        # ---- round 0 projections (f16 fixup path + fp8 token tile 1) ----
        qps = s_ps.tile([P, 512], f32, tag="sps", name="qp_0")
        proj16(0, xq16_sb, qps)
        nc.vector.tensor_copy(qt_all[:, 0:512], qps[:])
        kps = s_ps.tile([P, 512], f32, tag="sps", name="kp_0")
        proj16(1, x16_sb, kps)
        nc.vector.tensor_copy(kt_all[:, 0:512], kps[:])
        vps = s_ps.tile([P, 512], f32, tag="sps", name="vp_0")
        proj16(2, x16_sb, vps)
        v_chain(vps, 0)

        # ---- rounds: attention tt with round tt+1's projections woven in
        # (token tile 1 rides in attention 0's queue: its kt/v blocks are
        # first read by attention 0's pairs 2-3, after the unit completes)
        for tt in range(N_QT):
            units = proj_units(tt + 1)
            if tt == 0:
                units = [lambda: k_unit(1), lambda: v_unit(1)] + units

            # ---- attention for q-tile tt ----
            qs = qt_all[:, tt * 512:(tt + 1) * 512]
            ot = o_ps.tile([P, 512], f32, tag="ops", name=f"ot_{tt}")
            lt = l_ps.tile([16, 512], f32, tag="lps", name=f"lt_{tt}")
            pts = pts_pool.tile([P, 512], f16, tag="pts", name=f"pts_{tt}")
            npair = 4 * tt + 4
            nbelow = 4 * tt
            s_tiles = [None] * npair

            def c0_of(u):
                return 0 if u < nbelow else 128 * (u - nbelow)

            def emit_scores(u):
                c0 = c0_of(u)
                s = s_ps.tile([P, 2, 512], f32, tag="sps", name=f"s_{tt}_{u}")
                for m in (0, 1):
                    kb = 2 * u + m
                    nc.tensor.matmul(
                        s[:, m, c0:512],
                        lhsT=kt_all[:, kb * P:(kb + 1) * P],
                        rhs=qs[:, c0:512], start=True, stop=True)
                if u >= nbelow:
                    d = u - nbelow
                    nc.vector.tensor_add(
                        s[:, :, c0:c0 + P], s[:, :, c0:c0 + P],
                        mask_sb[:, 2 * d:2 * d + 2, :])
                s_tiles[u] = s

            emit_scores(0)
            for u in range(npair):
                if u + 1 < npair:
                    emit_scores(u + 1)
                s = s_tiles[u]
                c0 = c0_of(u)
                if u < nbelow or tt == N_QT - 1:
                    # fp8 path; the last tile's diagonal also runs fp8
                    # (its rows are global >=3072 so quantization washes
                    # out) which drops the PTS chain from the kernel tail
                    d = u - nbelow
                    last = (tt == N_QT - 1 and u == npair - 1)
                    pt = pt8_pool.tile([P, 2, 512], f8, tag="pt8")
                    nc.scalar.activation(pt[:, :, c0:512], s[:, :, c0:512], EXP,
                                         bias=bias_sb[:], scale=SCALE_ACT)
                    kb2 = 2 * u if u < nbelow else 8 * tt + 2 * d
                    nc.tensor.matmul(
                        ot[:, c0:512], lhsT=v8[:, kb2:kb2 + 2, :],
                        rhs=pt[:, :, c0:512],
                        start=(u == 0), stop=last, perf_mode=DR)
                    nc.tensor.matmul(
                        lt[:, c0:512], lhsT=ones8[:], rhs=pt[:, :, c0:512],
                        start=(u == 0), stop=last, perf_mode=DR)
                else:
                    d = u - nbelow
                    pt = pt16_pool.tile([P, 2, 512], f16, tag="pt16")
                    nc.scalar.activation(pt[:, :, c0:512], s[:, :, c0:512], EXP,
                                         bias=bias_sb[:], scale=SCALE_ACT)
                    for m in (0, 1):
                        nc.tensor.matmul(
                            ot[:, c0:512],
                            lhsT=v16[:, 8 * tt + 2 * d + m, :],
                            rhs=pt[:, m, c0:512],
                            start=(u == 0 and m == 0),
                            stop=(u == npair - 1 and m == 1))
                    if d == 0:
                        nc.vector.tensor_add(pts[:], pt[:, 0, :], pt[:, 1, :])
                    else:
                        tmp = ptt_pool.tile([P, 512], f16, tag="ptt")
                        nc.vector.tensor_add(tmp[:, c0:512], pt[:, 0, c0:512],
                                             pt[:, 1, c0:512])
                        nc.vector.tensor_add(pts[:, c0:512], pts[:, c0:512],
                                             tmp[:, c0:512])
                if units:
                    units.pop(0)()
            while units:
                units.pop(0)()
            if tt < N_QT - 1:
                nc.tensor.matmul(lt[:], lhsT=ones16[:], rhs=pts[:],
                                 start=(tt == 0), stop=True)

            # epilogue: PSUM -> SBUF -> HBM (normalize + transpose on host)
            o_sb = osb_pool.tile([P, 512], f32, tag="osb")
            nc.vector.tensor_copy(o_sb[:], ot[:])
            nc.vector.tensor_copy(l_all[0:1, tt, :], lt[0:1, :])
            nc.sync.dma_start(oT[tt, :, :], o_sb[:])
        nc.sync.dma_start(lsum[:, :], l_all[0:1, :, :])


def build_program():
    import concourse.tile as tile
    from concourse import bacc, mybir

    f32 = mybir.dt.float32
    f16 = mybir.dt.float16
    f8 = mybir.dt.float8e4
    nc = bacc.Bacc("TRN2", target_bir_lowering=False, debug=False,
                   num_devices=N_CORES)
    x16p = nc.dram_tensor("x16p", [P, NB_E, 512], f16, kind="ExternalInput").ap()
    xq16p = nc.dram_tensor("xq16p", [P, NB_E, 512], f16, kind="ExternalInput").ap()
    x8p = nc.dram_tensor("x8p", [4, P, 2, W8], f8, kind="ExternalInput").ap()
    xq8p = nc.dram_tensor("xq8p", [4, P, 2, WQ8], f8, kind="ExternalInput").ap()
    w16p = nc.dram_tensor("w16p", [P, 3, NB_E, P], f16, kind="ExternalInput").ap()
    w8p = nc.dram_tensor("w8p", [P, 3, NB_E, P], f8, kind="ExternalInput").ap()
    maskp = nc.dram_tensor("maskp", [P, 8, P], f16, kind="ExternalInput").ap()
    oT = nc.dram_tensor("oT", [N_QT, P, 512], f32, kind="ExternalOutput").ap()
    lsum = nc.dram_tensor("lsum", [N_QT, 512], f32, kind="ExternalOutput").ap()

    with tile.TileContext(nc) as tc:
        _emit(tc, (x16p, xq16p, x8p, xq8p, w16p, w8p, maskp, oT, lsum))
    nc.compile()
    return nc


def make_in_maps(x, Wq, Wk, Wv):
    """Per-core input maps. x: [B,T,E] f32; W*: [H,E] f32."""
    x = np.asarray(x, dtype=F32)
    # weights: [E, H] scaled, partition-major [P, 3, NB_E, P]
    w16p = np.empty((P, 3, NB_E, P), dtype=np.float16)
    w8p = np.empty((P, 3, NB_E, P), dtype=F8NP)
    for iw, W in enumerate((Wq, Wk, Wv)):
        wt = (np.asarray(W, dtype=F32).T * WSC)          # [E, H]
        wt = wt.reshape(NB_E, P, H).transpose(1, 0, 2)   # [P, NB_E, H]
        w16p[:, iw] = wt.astype(np.float16)
        w8p[:, iw] = wt.astype(F8NP)

    def pair_pack(arr, width):
        """[E, width] -> [4, P, 2, width]: chunk-pair partition-major."""
        return np.ascontiguousarray(
            arr.reshape(4, 2, P, width).transpose(0, 2, 1, 3))

    in_maps = []
    for c in range(N_CORES):
        b, p = c // 2, c % 2
        xb = x[b]                                       # [T, E]
        xT = np.ascontiguousarray(xb.T)                 # [E, T]
        xq = np.ascontiguousarray(xb[_query_rows(p)].T)  # [E, TQ]
        x16 = xT[:, :512].reshape(NB_E, P, 512)
        xq16 = xq[:, :512].reshape(NB_E, P, 512)
        in_maps.append({
            "x16p": np.ascontiguousarray(x16.transpose(1, 0, 2).astype(np.float16)),
            "xq16p": np.ascontiguousarray(xq16.transpose(1, 0, 2).astype(np.float16)),
            "x8p": pair_pack(np.ascontiguousarray(xT[:, 512:]).astype(F8NP), W8),
            "xq8p": pair_pack(np.ascontiguousarray(xq[:, 512:]).astype(F8NP), WQ8),
            "w16p": w16p, "w8p": w8p,
            "maskp": _mask_compact(p),
        })
    return in_maps


def postprocess(core_out):
    """Device outputs -> [TQ, H] f32 in gathered-row order."""
    oT = np.asarray(core_out["oT"], dtype=F32)      # [4, 128, 512]
    l = np.asarray(core_out["lsum"], dtype=F32)     # [4, 512]
    out = np.empty((TQ, H), dtype=F32)
    for t in range(N_QT):
        out[t * 512:(t + 1) * 512] = (oT[t] / l[t][None, :]).T / WSC
    return out


def _enable_ldw_opt():
    """Walrus ships with --enable-ldw-opt=false; the optimization overlaps
    LDWEIGHTS with the preceding matmul stream (weight double-buffering),
    which otherwise serializes ~100ns per matmul on the PE."""
    import concourse.bass_utils as bu
    if getattr(bu, "_ldw_patched", False):
        return
    bu._ldw_patched = True  # ldw-opt is incompatible with explicit
    # Ldweights codegen in this toolchain (walrus rejects it); LDWEIGHTS
    # already overlaps matmuls via the weight double-buffer.


def run(x, Wq, Wk, Wv, trace=False, trace_cores=None):
    """Returns (full_output [B,T,H] f32, BassKernelResults)."""
    from concourse.bass_utils import run_bass_kernel_spmd

    _enable_ldw_opt()
    nc = build_program()
    in_maps = make_in_maps(x, Wq, Wk, Wv)
    res = run_bass_kernel_spmd(
        nc, in_maps, list(range(N_CORES)), trace=trace,
        trace_cores=trace_cores,
    )
    full = np.empty((B, T, H), dtype=F32)
    for c in range(N_CORES):
        b, p = c // 2, c % 2
        full[b, _query_rows(p), :] = postprocess(res.results[c])
    return full, res


def kernel(x, Wq, Wk, Wv):
    full, _ = run(x, Wq, Wk, Wv, trace=False)
    return full


if __name__ == "__main__":
    nc = build_program()
    print("program built ok")
